# revision 17
# baseline (speedup 1.0000x reference)
"""Multi-Latent Attention TRN2 kernel — hybrid sharding.

8 cores = 2 batch groups x 4 cores. Each core handles ONE batch (2048
tokens) and 4 of the 16 heads. Host sums the 4 partials per batch and adds
the output bias. Identical FLOPs per core to pure head-parallel, but HALF
the HBM traffic (inputs are one batch: 3 x 8.4MB, partial out 8.4MB), so
the projection stream never outruns DMA and chip-level HBM pressure drops.

Device dataflow identical to the head-parallel kernel (feature-major S^T
attention with ones-matmul rowsums); heads are processed in pairs inside
each q-block so the PSUM budget (8 banks) still fits:
  'st' pool 2 x [128,2,512] (scores pairs / q-proj / out-proj)
  'acc' pool 4 x [128,512]  (latents, recon, rowsum + U accumulators)
Step i = {prefetch x DMAs for chunk i+1, projection of chunk i, attention
for q-block Q=i (head pairs 0 then 1)}.
"""

import math
from contextlib import ExitStack

import numpy as np

import concourse.mybir as mybir
from concourse import bacc
from concourse.bass import ds, ts
from concourse.tile import TileContext

B, S, D = 2, 2048, 2048
H, DK, DV, L = 16, 128, 128, 64
N_CORES = 8
GPB = N_CORES // B        # cores per batch group = 4
HPC = H // GPB            # heads per core = 4
NHP = HPC // 2            # head pairs = 2
SB = S                    # tokens per core's batch = 2048
T = B * S
FPC = HPC * DK            # feature cols per core = 512
LPC = HPC * L             # latent cols per core = 256
KO = D // 128             # 16
KG = 4
NG = KO // KG             # 4
QT = SB // 128            # 16
CHUNK = 512
NCH = SB // CHUNK         # 4

F32 = mybir.dt.float32
F32R = mybir.dt.float32r
BF16 = mybir.dt.bfloat16
IN_DT = BF16
OUT_DT = BF16

INV_SQRT_DK = 1.0 / math.sqrt(DK)
EXPF = mybir.ActivationFunctionType.Exp
IDF = mybir.ActivationFunctionType.Identity


def build_kernel():
    nc = bacc.Bacc(trn_type="TRN2", debug=False, num_swdge_queues=2)

    qT = nc.dram_tensor("qT", [D, SB], IN_DT, kind="ExternalInput")
    kT = nc.dram_tensor("kT", [D, SB], IN_DT, kind="ExternalInput")
    vT = nc.dram_tensor("vT", [D, SB], IN_DT, kind="ExternalInput")
    wq = nc.dram_tensor("wq", [D, FPC], IN_DT, kind="ExternalInput")
    bq = nc.dram_tensor("bq", [FPC], F32, kind="ExternalInput")
    wlk = nc.dram_tensor("wlk", [D, LPC], IN_DT, kind="ExternalInput")
    blk = nc.dram_tensor("blk", [LPC], F32, kind="ExternalInput")
    wlv = nc.dram_tensor("wlv", [D, LPC], IN_DT, kind="ExternalInput")
    blv = nc.dram_tensor("blv", [LPC], F32, kind="ExternalInput")
    wkr2 = nc.dram_tensor("wkr2", [128, 256], F32R, kind="ExternalInput")
    bkr = nc.dram_tensor("bkr", [DK], F32, kind="ExternalInput")
    wvr2 = nc.dram_tensor("wvr2", [128, 256], F32R, kind="ExternalInput")
    bvr = nc.dram_tensor("bvr", [DV], F32, kind="ExternalInput")
    wo = nc.dram_tensor("wo", [FPC, D], BF16, kind="ExternalInput")
    outp = nc.dram_tensor("outp", [SB, D], OUT_DT, kind="ExternalOutput")

    with TileContext(nc) as tc, ExitStack() as ctx:
        ec = ctx.enter_context
        consts = ec(tc.tile_pool(name="consts", bufs=1))
        persist = ec(tc.tile_pool(name="persist", bufs=1))
        xpool = ec(tc.tile_pool(name="xpool", bufs=12))
        latpool = ec(tc.tile_pool(name="latpool", bufs=6))
        ptpool = ec(tc.tile_pool(name="ptpool", bufs=6))
        statpool = ec(tc.tile_pool(name="statpool", bufs=4))
        opool = ec(tc.tile_pool(name="opool", bufs=2))
        psA = ec(tc.tile_pool(name="psA", bufs=2, space="PSUM"))
        psB = ec(tc.tile_pool(name="psB", bufs=4, space="PSUM"))

        qT_r = qT.rearrange("(ko p) t -> p ko t", p=128)
        kT_r = kT.rearrange("(ko p) t -> p ko t", p=128)
        vT_r = vT.rearrange("(ko p) t -> p ko t", p=128)

        def emit_xdma(c):
            t0 = c * CHUNK
            tiles = {}
            for g in range(NG):
                xqt = xpool.tile([128, KG, CHUNK], IN_DT, tag="x",
                                 name=f"xq{c}{g}")
                nc.sync.dma_start(xqt, qT_r[:, ds(g * KG, KG), ds(t0, CHUNK)])
                tiles[("q", g)] = xqt
            for g in range(NG):
                xkt = xpool.tile([128, KG, CHUNK], IN_DT, tag="x",
                                 name=f"xk{c}{g}")
                nc.sync.dma_start(xkt, kT_r[:, ds(g * KG, KG), ds(t0, CHUNK)])
                tiles[("k", g)] = xkt
            for g in range(NG):
                xvt = xpool.tile([128, KG, CHUNK], IN_DT, tag="x",
                                 name=f"xv{c}{g}")
                nc.gpsimd.dma_start(xvt, vT_r[:, ds(g * KG, KG), ds(t0, CHUNK)])
                tiles[("v", g)] = xvt
            return tiles

        # ---- weights / constants; wq + first q groups first for startup ----
        wq_r = wq.rearrange("(ko p) m -> p ko m", p=128)
        wq_sb = consts.tile([128, KO, FPC], IN_DT, tag="wq")
        x0 = {}
        for g in range(NG):
            nc.gpsimd.dma_start(
                wq_sb[:, ds(g * KG, KG), :], wq_r[:, ds(g * KG, KG), :])
            xqt = xpool.tile([128, KG, CHUNK], IN_DT, tag="x",
                             name=f"xq_0{g}")
            nc.scalar.dma_start(xqt, qT_r[:, ds(g * KG, KG), ds(0, CHUNK)])
            x0[("q", g)] = xqt
        bq_sb = consts.tile([128, HPC], F32, tag="bq")
        nc.gpsimd.dma_start(bq_sb, bq.rearrange("(m p) -> p m", p=128))

        wlk_sb = consts.tile([128, KO, LPC], IN_DT, tag="wlk")
        nc.scalar.dma_start(wlk_sb, wlk.rearrange("(ko p) m -> p ko m", p=128))
        blk_sb = consts.tile([128, 2], F32, tag="blk")
        nc.gpsimd.dma_start(blk_sb, blk.rearrange("(m p) -> p m", p=128))
        for g in range(NG):
            xkt = xpool.tile([128, KG, CHUNK], IN_DT, tag="x",
                             name=f"xk_0{g}")
            nc.sync.dma_start(xkt, kT_r[:, ds(g * KG, KG), ds(0, CHUNK)])
            x0[("k", g)] = xkt

        wlv_sb = consts.tile([128, KO, LPC], IN_DT, tag="wlv")
        nc.scalar.dma_start(wlv_sb, wlv.rearrange("(ko p) m -> p ko m", p=128))
        blv_sb = consts.tile([128, 2], F32, tag="blv")
        nc.gpsimd.dma_start(blv_sb, blv.rearrange("(m p) -> p m", p=128))
        for g in range(NG):
            xvt = xpool.tile([128, KG, CHUNK], IN_DT, tag="x",
                             name=f"xv_0{g}")
            nc.gpsimd.dma_start(xvt, vT_r[:, ds(g * KG, KG), ds(0, CHUNK)])
            x0[("v", g)] = xvt

        wkr2_sb = consts.tile([128, 256], F32R, tag="wkr2")
        nc.gpsimd.dma_start(wkr2_sb, wkr2[:, :])
        wvr2_sb = consts.tile([128, 256], F32R, tag="wvr2")
        nc.gpsimd.dma_start(wvr2_sb, wvr2[:, :])
        bkr_sb = consts.tile([128, 1], F32, tag="bkr")
        nc.gpsimd.dma_start(bkr_sb, bkr[:, None])

        maskT = consts.tile([128, 128], BF16, tag="maskT")
        nc.gpsimd.memset(maskT, 1.0)
        nc.gpsimd.affine_select(
            out=maskT, in_=maskT, compare_op=mybir.AluOpType.is_ge,
            fill=0.0, base=0, pattern=[[1, 128]], channel_multiplier=-1,
        )
        ones_bf = consts.tile([128, 128], BF16, tag="ones_bf")
        nc.gpsimd.memset(ones_bf, 1.0)

        wo_sb = consts.tile([128, HPC, D], BF16, tag="wo")
        nc.gpsimd.dma_start(wo_sb, wo.rearrange("(kk p) d -> p kk d", p=128))

        asb = persist.tile([128, HPC, SB], BF16, tag="asb")
        qsb = persist.tile([128, HPC, SB], BF16, tag="qsb")
        ksb = persist.tile([128, HPC, SB], BF16, tag="ksb")
        vsb = persist.tile([128, QT, FPC], BF16, tag="vsb")

        xcur = x0
        for c in range(NCH):
            xnext = emit_xdma(c + 1) if c + 1 < NCH else None
            csl = ds(c * CHUNK, CHUNK)

            # ---- q projection: 4 heads -> two [128,2,512] psum tiles ----
            stqs = [psA.tile([128, 2, 512], F32, tag="st", name=f"stq{hp}")
                    for hp in range(NHP)]
            for ko in range(KO):
                for m in range(HPC):
                    nc.tensor.matmul(
                        stqs[m // 2][:, m % 2, :],
                        wq_sb[:, ko, ts(m, 128)],
                        xcur[("q", ko // KG)][:, ko % KG, :],
                        start=(ko == 0), stop=(ko == KO - 1),
                    )
            for m in range(HPC):
                nc.scalar.activation(
                    qsb[:, m, csl], stqs[m // 2][:, m % 2, :], IDF,
                    bias=bq_sb[:, m : m + 1])

            # ---- latk (2 latent tiles) -> k^T per head ----
            lks = []
            psls = [psB.tile([128, 512], F32, tag="acc", name=f"psl{lt}")
                    for lt in range(2)]
            for ko in range(KO):
                for lt in range(2):
                    nc.tensor.matmul(
                        psls[lt], wlk_sb[:, ko, ts(lt, 128)],
                        xcur[("k", ko // KG)][:, ko % KG, :],
                        start=(ko == 0), stop=(ko == KO - 1),
                    )
            for lt in range(2):
                lk = latpool.tile([128, 512], F32R, tag="lat",
                                  name=f"lk{lt}")
                nc.scalar.activation(lk, psls[lt], IDF,
                                     bias=blk_sb[:, lt : lt + 1])
                lks.append(lk)
            for h in range(HPC):
                psk = psB.tile([128, 512], F32, tag="acc")
                nc.tensor.matmul(
                    psk, wkr2_sb[:, ts(h % 2, 128)], lks[h // 2],
                    start=True, stop=True)
                nc.vector.tensor_scalar_add(
                    ksb[:, h, csl], psk, bkr_sb[:, 0:1])

            # ---- latv -> v (token-major, 512 feature cols) ----
            lvs = []
            psvs = [psB.tile([128, 512], F32, tag="acc", name=f"psv{lt}")
                    for lt in range(2)]
            for ko in range(KO):
                for lt in range(2):
                    nc.tensor.matmul(
                        psvs[lt], wlv_sb[:, ko, ts(lt, 128)],
                        xcur[("v", ko // KG)][:, ko % KG, :],
                        start=(ko == 0), stop=(ko == KO - 1),
                    )
            for lt in range(2):
                lv = latpool.tile([128, 512], F32R, tag="lat",
                                  name=f"lv{lt}")
                nc.scalar.activation(lv, psvs[lt], IDF,
                                     bias=blv_sb[:, lt : lt + 1])
                lvs.append(lv)
            for j2 in range(4):  # 128-token tiles of this chunk
                psu2 = psB.tile([128, 512], F32, tag="acc")
                for lt in range(2):
                    nc.tensor.matmul(
                        psu2[:, ts(lt, 256)],
                        lvs[lt][:, ts(j2, 128)], wvr2_sb,
                        start=True, stop=True,
                    )
                jt = c * 4 + j2
                nc.any.tensor_copy(out=vsb[:, jt, :], in_=psu2)

            # ---- attention for q-block Q = c, head pairs ----
            Q = c
            jmax = 4 * Q + 4
            for hp in range(NHP):
                h0 = 2 * hp
                o_acc = [psB.tile([128, 512], F32, tag="acc",
                                  name=f"o_acc{h}") for h in range(2)]
                u_acc = [psB.tile([128, 512], F32, tag="acc",
                                  name=f"u_acc{h}") for h in range(2)]
                pts = [None] * jmax

                def emit_ou(j):
                    qoff, pt = pts[j]
                    for hh in range(2):
                        nc.tensor.matmul(
                            o_acc[hh][:, qoff:], ones_bf, pt[:, hh, qoff:],
                            start=(j == 0), stop=(j == jmax - 1),
                        )
                        nc.tensor.matmul(
                            u_acc[hh][:, qoff:],
                            vsb[:, j, ts(h0 + hh, 128)],
                            pt[:, hh, qoff:],
                            start=(j == 0), stop=(j == jmax - 1),
                        )

                for j in range(jmax):
                    qoff = max(0, (j - 4 * Q) * 128)
                    n = 512 - qoff
                    st = psA.tile([128, 2, 512], F32, tag="st")
                    for hh in range(2):
                        nc.tensor.matmul(
                            st[:, hh, qoff:], ksb[:, h0 + hh, ts(j, 128)],
                            qsb[:, h0 + hh, ds(Q * 512 + qoff, n)],
                            start=True, stop=True,
                        )
                    pt = ptpool.tile([128, 2, 512], BF16, tag="pt")
                    nc.scalar.activation(
                        pt[:, :, qoff:], st[:, :, qoff:],
                        EXPF, scale=INV_SQRT_DK,
                    )
                    if j >= 4 * Q:
                        for hh in range(2):
                            nc.vector.tensor_tensor(
                                pt[:, hh, ds(qoff, 128)],
                                pt[:, hh, ds(qoff, 128)],
                                maskT, mybir.AluOpType.mult,
                            )
                    pts[j] = (qoff, pt)
                    if j > 0:
                        emit_ou(j - 1)
                emit_ou(jmax - 1)

                for hh in range(2):
                    rcp_sb = statpool.tile([128, 512], F32, tag="rcp")
                    nc.vector.reciprocal_approx_fast(rcp_sb, o_acc[hh])
                    a_sl = asb[:, h0 + hh, ds(Q * 512, 512)]
                    nc.vector.tensor_tensor(a_sl, u_acc[hh], rcp_sb,
                                            mybir.AluOpType.mult)

            # ---- out-projection for this q-block ----
            for tl in range(4):
                tt = Q * 4 + tl
                o_sb = opool.tile([128, D], OUT_DT, tag="o")
                for dc2 in range(2):
                    ps_f = psA.tile([128, 2, 512], F32, tag="st")
                    for half in range(2):
                        dc = dc2 * 2 + half
                        for kk in range(HPC):
                            nc.tensor.matmul(
                                ps_f[:, half, :],
                                asb[:, kk, ts(tt, 128)],
                                wo_sb[:, kk, ts(dc, 512)],
                                start=(kk == 0), stop=(kk == HPC - 1),
                            )
                    nc.any.tensor_copy(
                        out=o_sb[:, ds(dc2 * 1024, 1024)].rearrange(
                            "p (a b) -> p a b", a=2),
                        in_=ps_f,
                    )
                if tl % 2 == 0:
                    nc.sync.dma_start(outp[ts(tt, 128), :], o_sb)
                else:
                    nc.gpsimd.dma_start(outp[ts(tt, 128), :], o_sb)

            xcur = xnext

    nc.finalize()
    return nc


_NC_CACHE = None


def _get_nc():
    global _NC_CACHE
    if _NC_CACHE is None:
        _NC_CACHE = build_kernel()
    return _NC_CACHE


def _prep_in_maps(queries, keys, values, Wq, bq, Wlk, blk, Wlv, blv,
                  Wkr, bkr, Wvr, bvr, Wo, bo):
    f = np.float32
    import ml_dtypes

    ind = ml_dtypes.bfloat16

    qTh = [np.ascontiguousarray(np.asarray(queries)[b].T.astype(ind))
           for b in range(B)]
    kTh = [np.ascontiguousarray(np.asarray(keys)[b].T.astype(ind))
           for b in range(B)]
    vTh = [np.ascontiguousarray(np.asarray(values)[b].T.astype(ind))
           for b in range(B)]

    wkr2 = np.zeros((128, 256), f)
    wkr2[0:L, 0:DK] = Wkr
    wkr2[L : 2 * L, DK : 2 * DK] = Wkr
    wvr2 = np.zeros((128, 256), f)
    wvr2[0:L, 0:DV] = Wvr
    wvr2[L : 2 * L, DV : 2 * DV] = Wvr

    in_maps = []
    for ci in range(N_CORES):
        b = ci // GPB
        hq = ci % GPB
        fsl = slice(hq * FPC, (hq + 1) * FPC)
        lsl = slice(hq * LPC, (hq + 1) * LPC)
        in_maps.append({
            "qT": qTh[b], "kT": kTh[b], "vT": vTh[b],
            "wq": np.ascontiguousarray(np.asarray(Wq)[:, fsl].astype(ind)),
            "bq": np.ascontiguousarray(np.asarray(bq)[fsl], f),
            "wlk": np.ascontiguousarray(np.asarray(Wlk)[:, lsl].astype(ind)),
            "blk": np.ascontiguousarray(np.asarray(blk)[lsl], f),
            "wlv": np.ascontiguousarray(np.asarray(Wlv)[:, lsl].astype(ind)),
            "blv": np.ascontiguousarray(np.asarray(blv)[lsl], f),
            "wkr2": wkr2, "bkr": np.ascontiguousarray(np.asarray(bkr), f),
            "wvr2": wvr2, "bvr": np.ascontiguousarray(np.asarray(bvr), f),
            "wo": np.ascontiguousarray(
                np.asarray(Wo)[fsl, :].astype(ml_dtypes.bfloat16)),
        })
    return in_maps


def _assemble(results, bias_row):
    out = np.zeros((B, S, D), np.float32)
    for b in range(B):
        acc = np.zeros((S, D), np.float64)
        for ci in range(b * GPB, (b + 1) * GPB):
            acc += results[ci]["outp"].astype(np.float64)
        out[b] = (acc + bias_row).astype(np.float32)
    return out


def kernel(**inputs):
    from concourse.bass_utils import run_bass_kernel_spmd

    nc = _get_nc()
    in_maps = _prep_in_maps(**inputs)
    res = run_bass_kernel_spmd(
        nc, in_maps, core_ids=list(range(N_CORES)), trace=False
    )
    bias_row = (np.tile(np.asarray(inputs["bvr"], np.float64), H)
                @ np.asarray(inputs["Wo"], np.float64)
                + np.asarray(inputs["bo"], np.float64))
    return _assemble(res.results, bias_row)


if __name__ == "__main__":
    nc = build_kernel()
    print("built ok, instructions:", len(nc.inst_map))


# revision 18
# speedup vs baseline: 1.1912x; 1.1912x over previous
"""Multi-Latent Attention TRN2 kernel — hybrid sharding.

8 cores = 2 batch groups x 4 cores. Each core handles ONE batch (2048
tokens) and 4 of the 16 heads. Host sums the 4 partials per batch and adds
the output bias. Identical FLOPs per core to pure head-parallel, but HALF
the HBM traffic (inputs are one batch: 3 x 8.4MB, partial out 8.4MB), so
the projection stream never outruns DMA and chip-level HBM pressure drops.

Device dataflow identical to the head-parallel kernel (feature-major S^T
attention with ones-matmul rowsums); heads are processed in pairs inside
each q-block so the PSUM budget (8 banks) still fits:
  'st' pool 2 x [128,2,512] (scores pairs / q-proj / out-proj)
  'acc' pool 4 x [128,512]  (latents, recon, rowsum + U accumulators)
Step i = {prefetch x DMAs for chunk i+1, projection of chunk i, attention
for q-block Q=i (head pairs 0 then 1)}.
"""

import math
from contextlib import ExitStack

import numpy as np

import concourse.mybir as mybir
from concourse import bacc
from concourse.bass import ds, ts
from concourse.tile import TileContext

B, S, D = 2, 2048, 2048
H, DK, DV, L = 16, 128, 128, 64
N_CORES = 8
GPB = N_CORES // B        # cores per batch group = 4
HPC = H // GPB            # heads per core = 4
NHP = HPC // 2            # head pairs = 2
SB = S                    # tokens per core's batch = 2048
T = B * S
FPC = HPC * DK            # feature cols per core = 512
LPC = HPC * L             # latent cols per core = 256
KO = D // 128             # 16
KG = 4
NG = KO // KG             # 4
QT = SB // 128            # 16
CHUNK = 512
NCH = SB // CHUNK         # 4

F32 = mybir.dt.float32
F32R = mybir.dt.float32r
BF16 = mybir.dt.bfloat16
IN_DT = BF16
OUT_DT = BF16

INV_SQRT_DK = 1.0 / math.sqrt(DK)
EXPF = mybir.ActivationFunctionType.Exp
IDF = mybir.ActivationFunctionType.Identity


def build_kernel():
    nc = bacc.Bacc(trn_type="TRN2", debug=False, num_swdge_queues=2)

    qT = nc.dram_tensor("qT", [D, SB], IN_DT, kind="ExternalInput")
    kT = nc.dram_tensor("kT", [D, SB], IN_DT, kind="ExternalInput")
    vT = nc.dram_tensor("vT", [D, SB], IN_DT, kind="ExternalInput")
    wq = nc.dram_tensor("wq", [D, FPC], IN_DT, kind="ExternalInput")
    bq = nc.dram_tensor("bq", [FPC], F32, kind="ExternalInput")
    wlk = nc.dram_tensor("wlk", [D, LPC], IN_DT, kind="ExternalInput")
    blk = nc.dram_tensor("blk", [LPC], F32, kind="ExternalInput")
    wlv = nc.dram_tensor("wlv", [D, LPC], IN_DT, kind="ExternalInput")
    blv = nc.dram_tensor("blv", [LPC], F32, kind="ExternalInput")
    wkr2 = nc.dram_tensor("wkr2", [128, 256], F32R, kind="ExternalInput")
    bkr = nc.dram_tensor("bkr", [DK], F32, kind="ExternalInput")
    wvr2 = nc.dram_tensor("wvr2", [128, 256], F32R, kind="ExternalInput")
    bvr = nc.dram_tensor("bvr", [DV], F32, kind="ExternalInput")
    wo = nc.dram_tensor("wo", [FPC, D], BF16, kind="ExternalInput")
    outp = nc.dram_tensor("outp", [SB, D], OUT_DT, kind="ExternalOutput")

    with TileContext(nc) as tc, ExitStack() as ctx:
        ec = ctx.enter_context
        consts = ec(tc.tile_pool(name="consts", bufs=1))
        persist = ec(tc.tile_pool(name="persist", bufs=1))
        xpool = ec(tc.tile_pool(name="xpool", bufs=12))
        latpool = ec(tc.tile_pool(name="latpool", bufs=6))
        ptpool = ec(tc.tile_pool(name="ptpool", bufs=6))
        statpool = ec(tc.tile_pool(name="statpool", bufs=4))
        opool = ec(tc.tile_pool(name="opool", bufs=2))
        psA = ec(tc.tile_pool(name="psA", bufs=2, space="PSUM"))
        psB = ec(tc.tile_pool(name="psB", bufs=4, space="PSUM"))

        qT_r = qT.rearrange("(ko p) t -> p ko t", p=128)
        kT_r = kT.rearrange("(ko p) t -> p ko t", p=128)
        vT_r = vT.rearrange("(ko p) t -> p ko t", p=128)

        def emit_xdma(c):
            t0 = c * CHUNK
            tiles = {}
            for g in range(NG):
                xqt = xpool.tile([128, KG, CHUNK], IN_DT, tag="x",
                                 name=f"xq{c}{g}")
                nc.sync.dma_start(xqt, qT_r[:, ds(g * KG, KG), ds(t0, CHUNK)])
                tiles[("q", g)] = xqt
            for g in range(NG):
                xkt = xpool.tile([128, KG, CHUNK], IN_DT, tag="x",
                                 name=f"xk{c}{g}")
                nc.sync.dma_start(xkt, kT_r[:, ds(g * KG, KG), ds(t0, CHUNK)])
                tiles[("k", g)] = xkt
            for g in range(NG):
                xvt = xpool.tile([128, KG, CHUNK], IN_DT, tag="x",
                                 name=f"xv{c}{g}")
                nc.gpsimd.dma_start(xvt, vT_r[:, ds(g * KG, KG), ds(t0, CHUNK)])
                tiles[("v", g)] = xvt
            return tiles

        # ---- weights / constants; wq + first q groups first for startup ----
        wq_r = wq.rearrange("(ko p) m -> p ko m", p=128)
        wq_sb = consts.tile([128, KO, FPC], IN_DT, tag="wq")
        x0 = {}
        for g in range(NG):
            nc.gpsimd.dma_start(
                wq_sb[:, ds(g * KG, KG), :], wq_r[:, ds(g * KG, KG), :])
            xqt = xpool.tile([128, KG, CHUNK], IN_DT, tag="x",
                             name=f"xq_0{g}")
            nc.scalar.dma_start(xqt, qT_r[:, ds(g * KG, KG), ds(0, CHUNK)])
            x0[("q", g)] = xqt
        bq_sb = consts.tile([128, HPC], F32, tag="bq")
        nc.gpsimd.dma_start(bq_sb, bq.rearrange("(m p) -> p m", p=128))

        wlk_sb = consts.tile([128, KO, LPC], IN_DT, tag="wlk")
        nc.gpsimd.dma_start(wlk_sb, wlk.rearrange("(ko p) m -> p ko m", p=128))
        blk_sb = consts.tile([128, 2], F32, tag="blk")
        nc.gpsimd.dma_start(blk_sb, blk.rearrange("(m p) -> p m", p=128))
        for g in range(NG):
            xkt = xpool.tile([128, KG, CHUNK], IN_DT, tag="x",
                             name=f"xk_0{g}")
            nc.sync.dma_start(xkt, kT_r[:, ds(g * KG, KG), ds(0, CHUNK)])
            x0[("k", g)] = xkt

        wlv_sb = consts.tile([128, KO, LPC], IN_DT, tag="wlv")
        nc.gpsimd.dma_start(wlv_sb, wlv.rearrange("(ko p) m -> p ko m", p=128))
        blv_sb = consts.tile([128, 2], F32, tag="blv")
        nc.gpsimd.dma_start(blv_sb, blv.rearrange("(m p) -> p m", p=128))
        for g in range(NG):
            xvt = xpool.tile([128, KG, CHUNK], IN_DT, tag="x",
                             name=f"xv_0{g}")
            nc.gpsimd.dma_start(xvt, vT_r[:, ds(g * KG, KG), ds(0, CHUNK)])
            x0[("v", g)] = xvt

        wkr2_sb = consts.tile([128, 256], F32R, tag="wkr2")
        nc.gpsimd.dma_start(wkr2_sb, wkr2[:, :])
        wvr2_sb = consts.tile([128, 256], F32R, tag="wvr2")
        nc.gpsimd.dma_start(wvr2_sb, wvr2[:, :])
        bkr_sb = consts.tile([128, 1], F32, tag="bkr")
        nc.gpsimd.dma_start(bkr_sb, bkr[:, None])

        maskT = consts.tile([128, 128], BF16, tag="maskT")
        nc.gpsimd.memset(maskT, 1.0)
        nc.gpsimd.affine_select(
            out=maskT, in_=maskT, compare_op=mybir.AluOpType.is_ge,
            fill=0.0, base=0, pattern=[[1, 128]], channel_multiplier=-1,
        )
        ones_bf = consts.tile([128, 128], BF16, tag="ones_bf")
        nc.gpsimd.memset(ones_bf, 1.0)

        wo_sb = consts.tile([128, HPC, D], BF16, tag="wo")
        nc.gpsimd.dma_start(wo_sb, wo.rearrange("(kk p) d -> p kk d", p=128))

        asb = persist.tile([128, HPC, SB], BF16, tag="asb")
        qsb = persist.tile([128, HPC, SB], BF16, tag="qsb")
        ksb = persist.tile([128, HPC, SB], BF16, tag="ksb")
        vsb = persist.tile([128, QT, FPC], BF16, tag="vsb")

        xcur = x0
        for c in range(NCH):
            xnext = emit_xdma(c + 1) if c + 1 < NCH else None
            csl = ds(c * CHUNK, CHUNK)

            # ---- q projection: 4 heads -> two [128,2,512] psum tiles ----
            stqs = [psA.tile([128, 2, 512], F32, tag="st", name=f"stq{hp}")
                    for hp in range(NHP)]
            for ko in range(KO):
                for m in range(HPC):
                    nc.tensor.matmul(
                        stqs[m // 2][:, m % 2, :],
                        wq_sb[:, ko, ts(m, 128)],
                        xcur[("q", ko // KG)][:, ko % KG, :],
                        start=(ko == 0), stop=(ko == KO - 1),
                    )
            for m in range(HPC):
                nc.scalar.activation(
                    qsb[:, m, csl], stqs[m // 2][:, m % 2, :], IDF,
                    bias=bq_sb[:, m : m + 1])

            # ---- latk (2 latent tiles) -> k^T per head ----
            lks = []
            psls = [psB.tile([128, 512], F32, tag="acc", name=f"psl{lt}")
                    for lt in range(2)]
            for ko in range(KO):
                for lt in range(2):
                    nc.tensor.matmul(
                        psls[lt], wlk_sb[:, ko, ts(lt, 128)],
                        xcur[("k", ko // KG)][:, ko % KG, :],
                        start=(ko == 0), stop=(ko == KO - 1),
                    )
            for lt in range(2):
                lk = latpool.tile([128, 512], F32R, tag="lat",
                                  name=f"lk{lt}")
                nc.scalar.activation(lk, psls[lt], IDF,
                                     bias=blk_sb[:, lt : lt + 1])
                lks.append(lk)
            for h in range(HPC):
                psk = psB.tile([128, 512], F32, tag="acc")
                nc.tensor.matmul(
                    psk, wkr2_sb[:, ts(h % 2, 128)], lks[h // 2],
                    start=True, stop=True)
                nc.vector.tensor_scalar_add(
                    ksb[:, h, csl], psk, bkr_sb[:, 0:1])

            # ---- latv -> v (token-major, 512 feature cols) ----
            lvs = []
            psvs = [psB.tile([128, 512], F32, tag="acc", name=f"psv{lt}")
                    for lt in range(2)]
            for ko in range(KO):
                for lt in range(2):
                    nc.tensor.matmul(
                        psvs[lt], wlv_sb[:, ko, ts(lt, 128)],
                        xcur[("v", ko // KG)][:, ko % KG, :],
                        start=(ko == 0), stop=(ko == KO - 1),
                    )
            for lt in range(2):
                lv = latpool.tile([128, 512], F32R, tag="lat",
                                  name=f"lv{lt}")
                nc.scalar.activation(lv, psvs[lt], IDF,
                                     bias=blv_sb[:, lt : lt + 1])
                lvs.append(lv)
            for j2 in range(4):  # 128-token tiles of this chunk
                psu2 = psB.tile([128, 512], F32, tag="acc")
                for lt in range(2):
                    nc.tensor.matmul(
                        psu2[:, ts(lt, 256)],
                        lvs[lt][:, ts(j2, 128)], wvr2_sb,
                        start=True, stop=True,
                    )
                jt = c * 4 + j2
                nc.any.tensor_copy(out=vsb[:, jt, :], in_=psu2)

            # ---- attention for q-block Q = c, head pairs ----
            Q = c
            jmax = 4 * Q + 4
            for hp in range(NHP):
                h0 = 2 * hp
                o_acc = [psB.tile([128, 512], F32, tag="acc",
                                  name=f"o_acc{h}") for h in range(2)]
                u_acc = [psB.tile([128, 512], F32, tag="acc",
                                  name=f"u_acc{h}") for h in range(2)]
                pts = [None] * jmax

                def emit_ou(j):
                    qoff, pt = pts[j]
                    for hh in range(2):
                        nc.tensor.matmul(
                            o_acc[hh][:, qoff:], ones_bf, pt[:, hh, qoff:],
                            start=(j == 0), stop=(j == jmax - 1),
                        )
                        nc.tensor.matmul(
                            u_acc[hh][:, qoff:],
                            vsb[:, j, ts(h0 + hh, 128)],
                            pt[:, hh, qoff:],
                            start=(j == 0), stop=(j == jmax - 1),
                        )

                for j in range(jmax):
                    qoff = max(0, (j - 4 * Q) * 128)
                    n = 512 - qoff
                    st = psA.tile([128, 2, 512], F32, tag="st")
                    for hh in range(2):
                        nc.tensor.matmul(
                            st[:, hh, qoff:], ksb[:, h0 + hh, ts(j, 128)],
                            qsb[:, h0 + hh, ds(Q * 512 + qoff, n)],
                            start=True, stop=True,
                        )
                    pt = ptpool.tile([128, 2, 512], BF16, tag="pt")
                    nc.scalar.activation(
                        pt[:, :, qoff:], st[:, :, qoff:],
                        EXPF, scale=INV_SQRT_DK,
                    )
                    if j >= 4 * Q:
                        for hh in range(2):
                            nc.vector.tensor_tensor(
                                pt[:, hh, ds(qoff, 128)],
                                pt[:, hh, ds(qoff, 128)],
                                maskT, mybir.AluOpType.mult,
                            )
                    pts[j] = (qoff, pt)
                    if j > 0:
                        emit_ou(j - 1)
                emit_ou(jmax - 1)

                for hh in range(2):
                    rcp_sb = statpool.tile([128, 512], F32, tag="rcp")
                    nc.vector.reciprocal_approx_fast(rcp_sb, o_acc[hh])
                    a_sl = asb[:, h0 + hh, ds(Q * 512, 512)]
                    nc.vector.tensor_tensor(a_sl, u_acc[hh], rcp_sb,
                                            mybir.AluOpType.mult)

            # ---- out-projection for this q-block ----
            for tl in range(4):
                tt = Q * 4 + tl
                o_sb = opool.tile([128, D], OUT_DT, tag="o")
                for dc2 in range(2):
                    ps_f = psA.tile([128, 2, 512], F32, tag="st")
                    for half in range(2):
                        dc = dc2 * 2 + half
                        for kk in range(HPC):
                            nc.tensor.matmul(
                                ps_f[:, half, :],
                                asb[:, kk, ts(tt, 128)],
                                wo_sb[:, kk, ts(dc, 512)],
                                start=(kk == 0), stop=(kk == HPC - 1),
                            )
                    nc.any.tensor_copy(
                        out=o_sb[:, ds(dc2 * 1024, 1024)].rearrange(
                            "p (a b) -> p a b", a=2),
                        in_=ps_f,
                    )
                if tl % 2 == 0:
                    nc.sync.dma_start(outp[ts(tt, 128), :], o_sb)
                else:
                    nc.gpsimd.dma_start(outp[ts(tt, 128), :], o_sb)

            xcur = xnext

    nc.finalize()
    return nc


_NC_CACHE = None


def _get_nc():
    global _NC_CACHE
    if _NC_CACHE is None:
        _NC_CACHE = build_kernel()
    return _NC_CACHE


def _prep_in_maps(queries, keys, values, Wq, bq, Wlk, blk, Wlv, blv,
                  Wkr, bkr, Wvr, bvr, Wo, bo):
    f = np.float32
    import ml_dtypes

    ind = ml_dtypes.bfloat16

    qTh = [np.ascontiguousarray(np.asarray(queries)[b].T.astype(ind))
           for b in range(B)]
    kTh = [np.ascontiguousarray(np.asarray(keys)[b].T.astype(ind))
           for b in range(B)]
    vTh = [np.ascontiguousarray(np.asarray(values)[b].T.astype(ind))
           for b in range(B)]

    wkr2 = np.zeros((128, 256), f)
    wkr2[0:L, 0:DK] = Wkr
    wkr2[L : 2 * L, DK : 2 * DK] = Wkr
    wvr2 = np.zeros((128, 256), f)
    wvr2[0:L, 0:DV] = Wvr
    wvr2[L : 2 * L, DV : 2 * DV] = Wvr

    in_maps = []
    for ci in range(N_CORES):
        b = ci // GPB
        hq = ci % GPB
        fsl = slice(hq * FPC, (hq + 1) * FPC)
        lsl = slice(hq * LPC, (hq + 1) * LPC)
        in_maps.append({
            "qT": qTh[b], "kT": kTh[b], "vT": vTh[b],
            "wq": np.ascontiguousarray(np.asarray(Wq)[:, fsl].astype(ind)),
            "bq": np.ascontiguousarray(np.asarray(bq)[fsl], f),
            "wlk": np.ascontiguousarray(np.asarray(Wlk)[:, lsl].astype(ind)),
            "blk": np.ascontiguousarray(np.asarray(blk)[lsl], f),
            "wlv": np.ascontiguousarray(np.asarray(Wlv)[:, lsl].astype(ind)),
            "blv": np.ascontiguousarray(np.asarray(blv)[lsl], f),
            "wkr2": wkr2, "bkr": np.ascontiguousarray(np.asarray(bkr), f),
            "wvr2": wvr2, "bvr": np.ascontiguousarray(np.asarray(bvr), f),
            "wo": np.ascontiguousarray(
                np.asarray(Wo)[fsl, :].astype(ml_dtypes.bfloat16)),
        })
    return in_maps


def _assemble(results, bias_row):
    out = np.zeros((B, S, D), np.float32)
    for b in range(B):
        acc = np.zeros((S, D), np.float64)
        for ci in range(b * GPB, (b + 1) * GPB):
            acc += results[ci]["outp"].astype(np.float64)
        out[b] = (acc + bias_row).astype(np.float32)
    return out


def kernel(**inputs):
    from concourse.bass_utils import run_bass_kernel_spmd

    nc = _get_nc()
    in_maps = _prep_in_maps(**inputs)
    res = run_bass_kernel_spmd(
        nc, in_maps, core_ids=list(range(N_CORES)), trace=False
    )
    bias_row = (np.tile(np.asarray(inputs["bvr"], np.float64), H)
                @ np.asarray(inputs["Wo"], np.float64)
                + np.asarray(inputs["bo"], np.float64))
    return _assemble(res.results, bias_row)


if __name__ == "__main__":
    nc = build_kernel()
    print("built ok, instructions:", len(nc.inst_map))
